# revision 15
# baseline (speedup 1.0000x reference)
"""Paged GQA decode attention (sparse_attention) on 8 TRN2 NeuronCores.

Slot-parallel streaming design: the page table is (in the graded problem) a
permutation of all 32768 cache slots, so the full K/V caches are read exactly
once.  Core i owns the contiguous slot range [i*4096, (i+1)*4096) and STREAMS
it sequentially at full HBM bandwidth in chunks laid out
[128 partitions, r rows, 1024] (16KB contiguous DRAM per partition per
descriptor at r=4; chunks ramp small->large->small to fill and drain the
pipeline quickly).  K streams on the SP HWDGE queue, V on the gpsimd SWDGE
queue so neither compute engine pays DMA-issue time.

Each core scores its slots against ALL 16 sequences' queries, multiplies by a
host-built per-(slot, seq) ownership weight (0/1 under the permutation;
counts for general inputs), and accumulates the partial attention numerator
and denominator in PSUM over the whole stream.  The 16 new-token k/v rows
arrive as a host-prebuilt bf16 tile (zero-padded, ones-column included) whose
mask rows inject each new token exactly once across the fleet; the streamed
mask row of a replaced slot is zeroed on the core that owns it.  No gathers,
no scatters -- one fully static program per core.  The host sums the per-core
partials (flash-decoding split-K combine) and normalizes.

Per-instruction HW profiling showed the tensor engine is cadence-bound at
roughly max(ldweights, matmul-stream) ~ 1ns/column, so the pipeline minimizes
PE columns moved:
  - everything on the PE is bf16: K tiles are converted f32->bf16 on the
    Vector engine and V tiles on the Activation engine, two groups ahead of
    their consumers (f32r narrow matmuls run at 4 cycles/row and f32r weight
    loads at ~2 cycles/column),
  - PV is flipped (lhsT = masked-P, 64-column weight loads; V is the moving
    operand) and V carries an appended ones-column per head so each PV
    matmul also emits that head's denominator -- no separate den matmul,
  - PSUM accumulators are memset once and PV runs start=stop=False
    (skip_group_check): TRN2 zeroes lazily per 2KB region on
    start_tensor_calc, so 8 per-head groups in one bank would clobber each
    other.
"""

import ml_dtypes
import numpy as np

# ---- problem constants (must match the harness's reference.py) ----
NUM_HEADS = 32
NUM_KV_HEADS = 8
HEAD_DIM = 128
BS = 16
KV_LEN = 2048
NUM_SLOTS = BS * KV_LEN          # 32768
D = NUM_KV_HEADS * HEAD_DIM      # 1024 (cache row width, f32)
SCALE = HEAD_DIM ** -0.5
N_CORES = 8
GROUP = NUM_HEADS // NUM_KV_HEADS  # 4

SLOTS_PER_CORE = NUM_SLOTS // N_CORES   # 4096
RMAX = 4                                # max DRAM rows per partition per chunk
# per-chunk rows/partition: small chunks at both ends for pipeline fill/drain
RS = [1, 1, 2, 4, 4, 4, 4, 4, 4, 2, 1, 1]
assert sum(RS) == SLOTS_PER_CORE // 128
NT = SLOTS_PER_CORE // 128              # 32 streamed 128-slot groups
T = NT + 1                              # + appended new-token group
QCOLS = NUM_KV_HEADS * BS * GROUP       # 512 score columns: (h, b, g)
H = NUM_KV_HEADS
PIPE = 3                                # conversions run PIPE groups ahead


def build_program(mask_4d=True):
    import concourse.bacc as bacc
    import concourse.mybir as mybir
    import concourse.tile as tile

    f32 = mybir.dt.float32
    f32r = mybir.dt.float32r
    bf16 = mybir.dt.bfloat16
    EXP = mybir.ActivationFunctionType.Exp
    MULT = mybir.AluOpType.mult

    nc = bacc.Bacc("TRN2", target_bir_lowering=False, debug=False,
                   enable_asserts=False, num_devices=N_CORES,
                   num_swdge_queues=1)

    kc = nc.dram_tensor("k_shard", [SLOTS_PER_CORE, D], f32r,
                        kind="ExternalInput").ap()
    vc = nc.dram_tensor("v_shard", [SLOTS_PER_CORE, D], f32r,
                        kind="ExternalInput").ap()
    knew_d = nc.dram_tensor("k_new16", [128, D], bf16,
                            kind="ExternalInput").ap()
    vnew_d = nc.dram_tensor("v_new16", [128, H * 129], bf16,
                            kind="ExternalInput").ap()
    qT_d = nc.dram_tensor("qT", [HEAD_DIM, QCOLS], bf16,
                          kind="ExternalInput").ap()
    mask_d = nc.dram_tensor("mask", [128, T * BS], bf16,
                            kind="ExternalInput").ap()
    ident_d = nc.dram_tensor("ident", [128, 128], bf16,
                             kind="ExternalInput").ap()
    # num packs PV and den: per head 129 cols = 128 d + 1 denominator
    num_d = nc.dram_tensor("num", [64, H * 129], f32,
                           kind="ExternalOutput").ap()

    chunks = []
    off = 0
    for r in RS:
        chunks.append((off, r))
        off += 128 * r

    with tile.TileContext(nc) as tc:
        with tc.tile_pool(name="const", bufs=1) as constp, \
             tc.tile_pool(name="kbuf", bufs=4) as kpool, \
             tc.tile_pool(name="vbuf", bufs=4) as vpool, \
             tc.tile_pool(name="k16", bufs=4) as k16p, \
             tc.tile_pool(name="ktsb", bufs=3) as ktp, \
             tc.tile_pool(name="psb", bufs=6) as ppool, \
             tc.tile_pool(name="outs", bufs=1) as outp, \
             tc.tile_pool(name="ps_kt", bufs=3, space="PSUM") as ps_kt, \
             tc.tile_pool(name="ps_s", bufs=2, space="PSUM") as ps_s, \
             tc.tile_pool(name="ps_pv", bufs=1, space="PSUM") as ps_pv:

            qt_sb = constp.tile([128, QCOLS], bf16)
            mask_sb = constp.tile([128, T * BS], bf16)
            ident = constp.tile([128, 128], bf16)
            knew16 = constp.tile([128, D], bf16)
            vnew16 = constp.tile([128, H, 129], bf16)

            def load_kvnew():
                nc.gpsimd.dma_start(knew16[:], knew_d)
                nc.gpsimd.dma_start(
                    vnew16[:].rearrange("p h d -> p (h d)"), vnew_d)

            # v16 rotation is persistent so the appended ones-columns
            # (denominator producers) are initialized exactly once
            v16 = [constp.tile([128, H, 129], bf16, name=f"v16_{i}")
                   for i in range(5)]
            for i in range(5):
                nc.vector.memset(v16[i][:, :, 128], 1.0)

            # pv[bg, h, 0:128] = numerator, pv[bg, h, 128] = denominator.
            # 129*4B per head: 3 heads fit a 2KB PSUM bank
            pvs = [ps_pv.tile([64, 3, 129], f32, name="pvA"),
                   ps_pv.tile([64, 3, 129], f32, name="pvB"),
                   ps_pv.tile([64, 2, 129], f32, name="pvC")]
            for t_ in pvs:
                nc.vector.memset(t_[:], 0.0)

            def pv_out(h):
                return pvs[h // 3][:, h % 3, :]

            def conv_k(kbuf, j):
                k16 = k16p.tile([128, D], bf16, tag="k16")
                nc.vector.tensor_copy(k16[:], kbuf[:, j, :])
                return k16

            def conv_v(vbuf, j, t):
                vv = v16[t % 5]
                nc.scalar.copy(
                    vv[:, :, 0:128],
                    vbuf[:, j, :].rearrange("p (h d) -> p h d", d=128))
                return vv

            def group_front(k16, t):
                """one 128-slot group (bf16 K tile k16, bf16 V+ones vv)."""
                ktps = ps_kt.tile([128, H, 128], bf16, tag="ktps")
                for h in range(H):
                    nc.tensor.transpose(
                        ktps[:, h, :], k16[:, h * 128:(h + 1) * 128],
                        ident[:])
                ktsb = ktp.tile([128, H, 128], bf16, tag="kt")
                nc.vector.tensor_copy(ktsb[:, 0:4, :], ktps[:, 0:4, :])
                nc.scalar.copy(ktsb[:, 4:8, :], ktps[:, 4:8, :])

                # scores[slot, (h,b,g)] = sum_d K[slot,d_h] Q[(b,g),d]
                scores = ps_s.tile([128, QCOLS], f32, tag="scores")
                for h in range(H):
                    nc.tensor.matmul(
                        out=scores[:, h * 64:(h + 1) * 64],
                        lhsT=ktsb[:, h, :],
                        rhs=qt_sb[:, h * 64:(h + 1) * 64],
                        start=True, stop=True)

                p_sb = ppool.tile([128, QCOLS], bf16, tag="p")
                nc.scalar.activation(p_sb[:], scores[:], EXP, scale=SCALE)

                # ownership weights: pm[slot, (h,b,g)] = p * mask[slot, b]
                pm = ppool.tile([128, QCOLS], bf16, tag="pm")
                mcols = mask_sb[:, t * BS:(t + 1) * BS]
                if mask_4d:
                    nc.vector.tensor_tensor(
                        out=pm[:].rearrange("p (h b g) -> p h b g",
                                            h=H, b=BS, g=GROUP),
                        in0=p_sb[:].rearrange("p (h b g) -> p h b g",
                                              h=H, b=BS, g=GROUP),
                        in1=mcols.unsqueeze(1).unsqueeze(3).broadcast_to(
                            [128, H, BS, GROUP]),
                        op=MULT)
                else:
                    for h in range(H):
                        nc.vector.tensor_tensor(
                            out=pm[:, h * 64:(h + 1) * 64].rearrange(
                                "p (b g) -> p b g", g=GROUP),
                            in0=p_sb[:, h * 64:(h + 1) * 64].rearrange(
                                "p (b g) -> p b g", g=GROUP),
                            in1=mcols.unsqueeze(2).broadcast_to(
                                [128, BS, GROUP]),
                            op=MULT)

                return pm

            def group_pv(pm, vv, t):
                # num[bg, h, d] += sum_slot pm[slot, bg] [V | 1][slot, d]
                for h in range(H):
                    nc.tensor.matmul(
                        out=pv_out(h),
                        lhsT=pm[:, h * 64:(h + 1) * 64],
                        rhs=vv[:, h, :],
                        start=False, stop=False, skip_group_check=True)
                    if t == T - 1:
                        if h % 2 == 0:
                            nc.vector.tensor_copy(onum[:, h, :], pv_out(h))
                        else:
                            nc.scalar.copy(onum[:, h, :], pv_out(h))

            # software pipeline: K conversions run KLEAD groups ahead of
            # compute, V conversions VLEAD ahead (emitted after each group's
            # ops so exp/mask never queue behind a conversion waiting on DMA)
            KLEAD, VLEAD = PIPE, 2
            chunk_iter = iter(chunks)
            avail = []          # (kbuf, vbuf, j) not yet converted
            ready_k = {}        # t -> k16
            ready_v = {}        # t -> vv
            n_k = 0
            n_v = 0

            onum = outp.tile([64, H, 129], f32)

            first_pump = [True]

            def pump():
                off, r = next(chunk_iter)
                kbuf = kpool.tile([128, RMAX, D], f32r, tag="k")
                vbuf = vpool.tile([128, RMAX, D], f32r, tag="v")
                ksrc = kc[off:off + 128 * r, :].rearrange(
                    "(p j) d -> p j d", j=r)
                vsrc = vc[off:off + 128 * r, :].rearrange(
                    "(p j) d -> p j d", j=r)
                nc.sync.dma_start(
                    kbuf[:, 0:r, :].rearrange("p j d -> p (j d)"),
                    ksrc.rearrange("p j d -> p (j d)"))
                if first_pump[0]:
                    nc.sync.dma_start(ident[:], ident_d)
                    nc.sync.dma_start(qt_sb[:], qT_d)
                nc.sync.dma_start(
                    vbuf[:, 0:r, :].rearrange("p j d -> p (j d)"),
                    vsrc.rearrange("p j d -> p (j d)"))
                if first_pump[0]:
                    first_pump[0] = False
                    nc.gpsimd.dma_start(mask_sb[:], mask_d)
                avail.extend((kbuf, vbuf, j) for j in range(r))
                return kbuf, vbuf

            def ensure(upto_k, upto_v):
                while n_k[0] <= min(upto_k, T - 1) or \
                        n_v[0] <= min(upto_v, T - 1):
                    if n_k[0] <= min(upto_k, T - 1):
                        tkk = n_k[0]
                        if tkk == NT:
                            load_kvnew()
                            ready_k[NT] = knew16
                        else:
                            while len(avail) <= tkk:
                                pump()
                            ready_k[tkk] = conv_k(avail[tkk][0], avail[tkk][2])
                        n_k[0] += 1
                    if n_v[0] <= min(upto_v, T - 1) and n_v[0] < n_k[0]:
                        tvv = n_v[0]
                        if tvv == NT:
                            ready_v[NT] = vnew16
                        else:
                            ready_v[tvv] = conv_v(avail[tvv][1],
                                                  avail[tvv][2], tvv)
                        n_v[0] += 1

            n_k = [0]
            n_v = [0]
            pms = {}
            ensure(KLEAD - 1, VLEAD - 1)
            for t in range(T):
                pms[t] = group_front(ready_k.pop(t), t)
                if t > 0:
                    group_pv(pms.pop(t - 1), ready_v.pop(t - 1), t - 1)
                ensure(t + KLEAD, t + VLEAD)
            group_pv(pms.pop(T - 1), ready_v.pop(T - 1), T - 1)

            nc.sync.dma_start(
                num_d, onum[:].rearrange("p h d -> p (h d)"))

    nc.compile()
    return nc


def shard_inputs(q, k, v, k_cache, v_cache, slot_mapping, page_indices):
    """Host-side sharding: contiguous zero-copy cache slices per core plus
    small index-derived tensors (masks, transposed queries, new-token tiles)."""
    q = np.ascontiguousarray(np.asarray(q, dtype=np.float32))
    k = np.ascontiguousarray(np.asarray(k, dtype=np.float32))
    v = np.ascontiguousarray(np.asarray(v, dtype=np.float32))
    k_cache = np.asarray(k_cache, dtype=np.float32)
    v_cache = np.asarray(v_cache, dtype=np.float32)
    slot_mapping = np.asarray(slot_mapping, dtype=np.int64).ravel()
    page_indices = np.asarray(page_indices, dtype=np.int64)

    # qT[d, (h, b, g)] = q[b, h*GROUP+g, d]
    qr = q.reshape(BS, NUM_KV_HEADS, GROUP, HEAD_DIM)
    qT = np.ascontiguousarray(
        qr.transpose(3, 1, 0, 2).reshape(HEAD_DIM, QCOLS)
    ).astype(ml_dtypes.bfloat16)

    # ownership weights: count[slot, b] = multiplicity of slot in seq b's pages
    count = np.zeros((NUM_SLOTS, BS), dtype=np.float32)
    np.add.at(count,
              (page_indices.ravel(),
               np.repeat(np.arange(BS), KV_LEN)),
              1.0)
    # new-token slots: reference scatters k/v rows there BEFORE the gather, so
    # the streamed (old) row must contribute nothing; the appended kvnew tile
    # re-injects each referencing (seq, count) exactly once fleet-wide.
    # With duplicate slot_mapping entries the last writer wins (jax .at[].set).
    final_writer = {}
    for j in range(BS):
        final_writer[int(slot_mapping[j])] = j
    newcnt = np.zeros((BS, BS), dtype=np.float32)   # [kvnew row j, seq b]
    for s, j in final_writer.items():
        newcnt[j, :] = count[s, :]
        count[s, :] = 0.0

    # appended tile data, host-converted to the device layouts
    knew = np.zeros((128, D), dtype=np.float32)
    knew[:BS] = k
    knew16 = knew.astype(ml_dtypes.bfloat16)
    vnew16 = np.zeros((128, H, 129), dtype=np.float32)
    vnew16[:BS, :, 0:128] = v.reshape(BS, H, HEAD_DIM)
    vnew16[:, :, 128] = 1.0
    vnew16 = np.ascontiguousarray(
        vnew16.reshape(128, H * 129)).astype(ml_dtypes.bfloat16)

    # streamed slot (group t from chunk (off, r), partition p, sub j)
    #   = off + p*r + j ; group index t advances j-major within a chunk
    perm = np.empty(SLOTS_PER_CORE, dtype=np.int64)
    gi = 0
    off = 0
    for r in RS:
        idx = off + np.arange(128)[:, None] * r + np.arange(r)[None, :]
        for j in range(r):
            perm[gi * 128:(gi + 1) * 128] = idx[:, j]
            gi += 1
        off += 128 * r

    in_maps = []
    for c in range(N_CORES):
        base = c * SLOTS_PER_CORE
        mcore = count[base:base + SLOTS_PER_CORE][perm]     # [4096, 16]
        m = np.zeros((128, T * BS), dtype=np.float32)
        m[:, :NT * BS] = (
            mcore.reshape(NT, 128, BS).transpose(1, 0, 2)
            .reshape(128, NT * BS))
        # new-token tile mask: kvnew row j handled by core j % N_CORES
        for j in range(BS):
            if j % N_CORES == c:
                m[j, NT * BS:(NT + 1) * BS] = newcnt[j]
        in_maps.append({
            "k_shard": k_cache[base:base + SLOTS_PER_CORE],
            "v_shard": v_cache[base:base + SLOTS_PER_CORE],
            "k_new16": knew16,
            "v_new16": vnew16,
            "qT": qT,
            "mask": m.astype(ml_dtypes.bfloat16),
            "ident": np.eye(128, dtype=np.float32).astype(ml_dtypes.bfloat16),
        })
    return in_maps


_PROGS = {}
last_results = None  # BassKernelResults of the most recent kernel() call


def kernel(q, k, v, k_cache, v_cache, slot_mapping, page_indices):
    global last_results
    from concourse.bass_utils import run_bass_kernel_spmd

    in_maps = shard_inputs(q, k, v, k_cache, v_cache, slot_mapping,
                           page_indices)
    if "prog" not in _PROGS:
        try:
            _PROGS["prog"] = build_program(mask_4d=True)
        except Exception:
            _PROGS["prog"] = build_program(mask_4d=False)
    res = run_bass_kernel_spmd(_PROGS["prog"], in_maps,
                               core_ids=list(range(N_CORES)))
    last_results = res

    acc = np.zeros((64, H, 129), dtype=np.float64)
    for c in range(N_CORES):
        acc += res.results[c]["num"].astype(np.float64).reshape(64, H, 129)
    num = acc[:, :, 0:128]                      # [(b,g), h, d]
    den = acc[:, :, 128]                        # [(b,g), h]
    o = num / den[:, :, None]
    o = o.reshape(BS, GROUP, NUM_KV_HEADS, HEAD_DIM)   # [b, g, h, d]
    out = o.transpose(0, 2, 1, 3).reshape(BS, NUM_HEADS * HEAD_DIM)
    return np.ascontiguousarray(out.astype(np.float32))


# revision 17
# speedup vs baseline: 1.0825x; 1.0825x over previous
"""Paged GQA decode attention (sparse_attention) on 8 TRN2 NeuronCores.

Slot-parallel streaming design: the page table is (in the graded problem) a
permutation of all 32768 cache slots, so the full K/V caches are read exactly
once.  Core i owns the contiguous slot range [i*4096, (i+1)*4096) and STREAMS
it sequentially at full HBM bandwidth in chunks laid out
[128 partitions, r rows, 1024] (16KB contiguous DRAM per partition per
descriptor at r=4; chunks ramp small->large->small to fill and drain the
pipeline quickly).  K streams on the SP HWDGE queue, V on the gpsimd SWDGE
queue so neither compute engine pays DMA-issue time.

Each core scores its slots against ALL 16 sequences' queries, multiplies by a
host-built per-(slot, seq) ownership weight (0/1 under the permutation;
counts for general inputs), and accumulates the partial attention numerator
and denominator in PSUM over the whole stream.  The 16 new-token k/v rows
arrive as a host-prebuilt bf16 tile (zero-padded, ones-column included) whose
mask rows inject each new token exactly once across the fleet; the streamed
mask row of a replaced slot is zeroed on the core that owns it.  No gathers,
no scatters -- one fully static program per core.  The host sums the per-core
partials (flash-decoding split-K combine) and normalizes.

Per-instruction HW profiling showed the tensor engine is cadence-bound at
roughly max(ldweights, matmul-stream) ~ 1ns/column, so the pipeline minimizes
PE columns moved:
  - everything on the PE is bf16: K tiles are converted f32->bf16 on the
    Vector engine and V tiles on the Activation engine, two groups ahead of
    their consumers (f32r narrow matmuls run at 4 cycles/row and f32r weight
    loads at ~2 cycles/column),
  - PV is flipped (lhsT = masked-P, 64-column weight loads; V is the moving
    operand) and V carries an appended ones-column per head so each PV
    matmul also emits that head's denominator -- no separate den matmul,
  - PSUM accumulators are memset once and PV runs start=stop=False
    (skip_group_check): TRN2 zeroes lazily per 2KB region on
    start_tensor_calc, so 8 per-head groups in one bank would clobber each
    other.
"""

import ml_dtypes
import numpy as np

# ---- problem constants (must match the harness's reference.py) ----
NUM_HEADS = 32
NUM_KV_HEADS = 8
HEAD_DIM = 128
BS = 16
KV_LEN = 2048
NUM_SLOTS = BS * KV_LEN          # 32768
D = NUM_KV_HEADS * HEAD_DIM      # 1024 (cache row width, f32)
SCALE = HEAD_DIM ** -0.5
N_CORES = 8
GROUP = NUM_HEADS // NUM_KV_HEADS  # 4

SLOTS_PER_CORE = NUM_SLOTS // N_CORES   # 4096
RMAX = 4                                # max DRAM rows per partition per chunk
# per-chunk rows/partition: small chunks at both ends for pipeline fill/drain
RS = [1, 1, 2, 4, 4, 4, 4, 4, 4, 2, 1, 1]
assert sum(RS) == SLOTS_PER_CORE // 128
NT = SLOTS_PER_CORE // 128              # 32 streamed 128-slot groups
T = NT + 1                              # + appended new-token group
QCOLS = NUM_KV_HEADS * BS * GROUP       # 512 score columns: (h, b, g)
H = NUM_KV_HEADS
PIPE = 3                                # conversions run PIPE groups ahead


def build_program(mask_4d=True):
    import concourse.bacc as bacc
    import concourse.mybir as mybir
    import concourse.tile as tile

    f32 = mybir.dt.float32
    f32r = mybir.dt.float32r
    bf16 = mybir.dt.bfloat16
    EXP = mybir.ActivationFunctionType.Exp
    MULT = mybir.AluOpType.mult

    nc = bacc.Bacc("TRN2", target_bir_lowering=False, debug=False,
                   enable_asserts=False, num_devices=N_CORES,
                   num_swdge_queues=1)

    kc = nc.dram_tensor("k_shard", [SLOTS_PER_CORE, D], f32r,
                        kind="ExternalInput").ap()
    vc = nc.dram_tensor("v_shard", [SLOTS_PER_CORE, D], f32r,
                        kind="ExternalInput").ap()
    knew_d = nc.dram_tensor("k_new16", [128, D], f32r,
                            kind="ExternalInput").ap()
    vnew_d = nc.dram_tensor("v_new16", [128, H * 129], bf16,
                            kind="ExternalInput").ap()
    qT_d = nc.dram_tensor("qT", [HEAD_DIM, QCOLS], bf16,
                          kind="ExternalInput").ap()
    mask_d = nc.dram_tensor("mask", [128, T * BS], bf16,
                            kind="ExternalInput").ap()
    ident_d = nc.dram_tensor("ident", [128, 128], f32r,
                             kind="ExternalInput").ap()
    # num packs PV and den: per head 129 cols = 128 d + 1 denominator
    num_d = nc.dram_tensor("num", [64, H * 129], f32,
                           kind="ExternalOutput").ap()

    chunks = []
    off = 0
    for r in RS:
        chunks.append((off, r))
        off += 128 * r

    with tile.TileContext(nc) as tc:
        with tc.tile_pool(name="const", bufs=1) as constp, \
             tc.tile_pool(name="kbuf", bufs=4) as kpool, \
             tc.tile_pool(name="vbuf", bufs=4) as vpool, \
             tc.tile_pool(name="ktsb", bufs=3) as ktp, \
             tc.tile_pool(name="psb", bufs=6) as ppool, \
             tc.tile_pool(name="outs", bufs=1) as outp, \
             tc.tile_pool(name="ps_kt", bufs=3, space="PSUM") as ps_kt, \
             tc.tile_pool(name="ps_s", bufs=2, space="PSUM") as ps_s, \
             tc.tile_pool(name="ps_pv", bufs=1, space="PSUM") as ps_pv:

            qt_sb = constp.tile([128, QCOLS], bf16)
            mask_sb = constp.tile([128, T * BS], bf16)
            ident = constp.tile([128, 128], f32r)
            knew16 = constp.tile([128, D], f32r)
            vnew16 = constp.tile([128, H, 129], bf16)

            def load_kvnew():
                nc.gpsimd.dma_start(knew16[:], knew_d)
                nc.gpsimd.dma_start(
                    vnew16[:].rearrange("p h d -> p (h d)"), vnew_d)

            # v16 rotation is persistent so the appended ones-columns
            # (denominator producers) are initialized exactly once
            v16 = [constp.tile([128, H, 129], bf16, name=f"v16_{i}")
                   for i in range(5)]
            for i in range(5):
                nc.vector.memset(v16[i][:, :, 128], 1.0)

            # pv[bg, h, 0:128] = numerator, pv[bg, h, 128] = denominator.
            # 129*4B per head: 3 heads fit a 2KB PSUM bank
            pvs = [ps_pv.tile([64, 3, 129], f32, name="pvA"),
                   ps_pv.tile([64, 3, 129], f32, name="pvB"),
                   ps_pv.tile([64, 2, 129], f32, name="pvC")]
            for t_ in pvs:
                nc.vector.memset(t_[:], 0.0)

            def pv_out(h):
                return pvs[h // 3][:, h % 3, :]

            def conv_v(vbuf, j, t):
                vv = v16[t % 5]
                src_v = vbuf[:, j, :].rearrange("p (h d) -> p h d", d=128)
                nc.scalar.copy(vv[:, 0:4, 0:128], src_v[:, 0:4, :])
                nc.vector.tensor_copy(vv[:, 4:8, 0:128], src_v[:, 4:8, :])
                return vv

            def group_front(ksrc, t):
                """one 128-slot group; K transposed straight from f32r,
                the PSUM->SBUF copies do the bf16 cast."""
                ktsb = ktp.tile([128, H, 128], bf16, tag="kt")
                for hg in range(2):
                    ktps = ps_kt.tile([128, 4, 128], f32r, tag="ktps")
                    for i in range(4):
                        h = hg * 4 + i
                        nc.tensor.transpose(
                            ktps[:, i, :],
                            ksrc[:, h * 128:(h + 1) * 128],
                            ident[:])
                    dst = ktsb[:, hg * 4:hg * 4 + 4, :]
                    if hg == 0:
                        nc.vector.tensor_copy(dst, ktps[:])
                    else:
                        nc.scalar.copy(dst, ktps[:])

                # scores[slot, (h,b,g)] = sum_d K[slot,d_h] Q[(b,g),d]
                scores = ps_s.tile([128, QCOLS], f32, tag="scores")
                for h in range(H):
                    nc.tensor.matmul(
                        out=scores[:, h * 64:(h + 1) * 64],
                        lhsT=ktsb[:, h, :],
                        rhs=qt_sb[:, h * 64:(h + 1) * 64],
                        start=True, stop=True)

                p_sb = ppool.tile([128, QCOLS], bf16, tag="p")
                nc.scalar.activation(p_sb[:], scores[:], EXP, scale=SCALE)

                # ownership weights: pm[slot, (h,b,g)] = p * mask[slot, b]
                pm = ppool.tile([128, QCOLS], bf16, tag="pm")
                mcols = mask_sb[:, t * BS:(t + 1) * BS]
                if mask_4d:
                    nc.vector.tensor_tensor(
                        out=pm[:].rearrange("p (h b g) -> p h b g",
                                            h=H, b=BS, g=GROUP),
                        in0=p_sb[:].rearrange("p (h b g) -> p h b g",
                                              h=H, b=BS, g=GROUP),
                        in1=mcols.unsqueeze(1).unsqueeze(3).broadcast_to(
                            [128, H, BS, GROUP]),
                        op=MULT)
                else:
                    for h in range(H):
                        nc.vector.tensor_tensor(
                            out=pm[:, h * 64:(h + 1) * 64].rearrange(
                                "p (b g) -> p b g", g=GROUP),
                            in0=p_sb[:, h * 64:(h + 1) * 64].rearrange(
                                "p (b g) -> p b g", g=GROUP),
                            in1=mcols.unsqueeze(2).broadcast_to(
                                [128, BS, GROUP]),
                            op=MULT)

                return pm

            def group_pv(pm, vv, t):
                # num[bg, h, d] += sum_slot pm[slot, bg] [V | 1][slot, d]
                for h in range(H):
                    nc.tensor.matmul(
                        out=pv_out(h),
                        lhsT=pm[:, h * 64:(h + 1) * 64],
                        rhs=vv[:, h, :],
                        start=False, stop=False, skip_group_check=True)
                    if t == T - 1:
                        if h % 2 == 0:
                            nc.vector.tensor_copy(onum[:, h, :], pv_out(h))
                        else:
                            nc.scalar.copy(onum[:, h, :], pv_out(h))

            # software pipeline: V conversions run VLEAD groups ahead of
            # compute (emitted after each group's ops so exp/mask never queue
            # behind a conversion waiting on DMA); K needs no conversion
            VLEAD = 2
            chunk_iter = iter(chunks)
            avail = []          # (kbuf, vbuf, j) per group
            ready_v = {}        # t -> vv

            onum = outp.tile([64, H, 129], f32)

            first_pump = [True]

            def pump():
                off, r = next(chunk_iter)
                kbuf = kpool.tile([128, RMAX, D], f32r, tag="k")
                vbuf = vpool.tile([128, RMAX, D], f32r, tag="v")
                ksrc = kc[off:off + 128 * r, :].rearrange(
                    "(p j) d -> p j d", j=r)
                vsrc = vc[off:off + 128 * r, :].rearrange(
                    "(p j) d -> p j d", j=r)
                nc.sync.dma_start(
                    kbuf[:, 0:r, :].rearrange("p j d -> p (j d)"),
                    ksrc.rearrange("p j d -> p (j d)"))
                if first_pump[0]:
                    nc.sync.dma_start(ident[:], ident_d)
                    nc.sync.dma_start(qt_sb[:], qT_d)
                nc.sync.dma_start(
                    vbuf[:, 0:r, :].rearrange("p j d -> p (j d)"),
                    vsrc.rearrange("p j d -> p (j d)"))
                if first_pump[0]:
                    first_pump[0] = False
                    nc.gpsimd.dma_start(mask_sb[:], mask_d)
                avail.extend((kbuf, vbuf, j) for j in range(r))
                return kbuf, vbuf

            def ensure(upto_v):
                while n_v[0] <= min(upto_v, T - 1):
                    tvv = n_v[0]
                    if tvv == NT:
                        load_kvnew()
                        ready_v[NT] = vnew16
                    else:
                        while len(avail) <= tvv:
                            pump()
                        ready_v[tvv] = conv_v(avail[tvv][1],
                                              avail[tvv][2], tvv)
                    n_v[0] += 1

            n_v = [0]
            pms = {}
            ensure(VLEAD - 1)
            for t in range(T):
                if t < NT:
                    while len(avail) <= t:
                        pump()
                    kb, _, jj = avail[t]
                    pms[t] = group_front(kb[:, jj, :], t)
                else:
                    pms[t] = group_front(knew16[:], t)
                if t > 0:
                    group_pv(pms.pop(t - 1), ready_v.pop(t - 1), t - 1)
                ensure(t + VLEAD)
            group_pv(pms.pop(T - 1), ready_v.pop(T - 1), T - 1)

            nc.sync.dma_start(
                num_d, onum[:].rearrange("p h d -> p (h d)"))

    nc.compile()
    return nc


def shard_inputs(q, k, v, k_cache, v_cache, slot_mapping, page_indices):
    """Host-side sharding: contiguous zero-copy cache slices per core plus
    small index-derived tensors (masks, transposed queries, new-token tiles)."""
    q = np.ascontiguousarray(np.asarray(q, dtype=np.float32))
    k = np.ascontiguousarray(np.asarray(k, dtype=np.float32))
    v = np.ascontiguousarray(np.asarray(v, dtype=np.float32))
    k_cache = np.asarray(k_cache, dtype=np.float32)
    v_cache = np.asarray(v_cache, dtype=np.float32)
    slot_mapping = np.asarray(slot_mapping, dtype=np.int64).ravel()
    page_indices = np.asarray(page_indices, dtype=np.int64)

    # qT[d, (h, b, g)] = q[b, h*GROUP+g, d]
    qr = q.reshape(BS, NUM_KV_HEADS, GROUP, HEAD_DIM)
    qT = np.ascontiguousarray(
        qr.transpose(3, 1, 0, 2).reshape(HEAD_DIM, QCOLS)
    ).astype(ml_dtypes.bfloat16)

    # ownership weights: count[slot, b] = multiplicity of slot in seq b's pages
    count = np.zeros((NUM_SLOTS, BS), dtype=np.float32)
    np.add.at(count,
              (page_indices.ravel(),
               np.repeat(np.arange(BS), KV_LEN)),
              1.0)
    # new-token slots: reference scatters k/v rows there BEFORE the gather, so
    # the streamed (old) row must contribute nothing; the appended kvnew tile
    # re-injects each referencing (seq, count) exactly once fleet-wide.
    # With duplicate slot_mapping entries the last writer wins (jax .at[].set).
    final_writer = {}
    for j in range(BS):
        final_writer[int(slot_mapping[j])] = j
    newcnt = np.zeros((BS, BS), dtype=np.float32)   # [kvnew row j, seq b]
    for s, j in final_writer.items():
        newcnt[j, :] = count[s, :]
        count[s, :] = 0.0

    # appended tile data, host-converted to the device layouts
    knew = np.zeros((128, D), dtype=np.float32)
    knew[:BS] = k
    knew16 = knew
    vnew16 = np.zeros((128, H, 129), dtype=np.float32)
    vnew16[:BS, :, 0:128] = v.reshape(BS, H, HEAD_DIM)
    vnew16[:, :, 128] = 1.0
    vnew16 = np.ascontiguousarray(
        vnew16.reshape(128, H * 129)).astype(ml_dtypes.bfloat16)

    # streamed slot (group t from chunk (off, r), partition p, sub j)
    #   = off + p*r + j ; group index t advances j-major within a chunk
    perm = np.empty(SLOTS_PER_CORE, dtype=np.int64)
    gi = 0
    off = 0
    for r in RS:
        idx = off + np.arange(128)[:, None] * r + np.arange(r)[None, :]
        for j in range(r):
            perm[gi * 128:(gi + 1) * 128] = idx[:, j]
            gi += 1
        off += 128 * r

    in_maps = []
    for c in range(N_CORES):
        base = c * SLOTS_PER_CORE
        mcore = count[base:base + SLOTS_PER_CORE][perm]     # [4096, 16]
        m = np.zeros((128, T * BS), dtype=np.float32)
        m[:, :NT * BS] = (
            mcore.reshape(NT, 128, BS).transpose(1, 0, 2)
            .reshape(128, NT * BS))
        # new-token tile mask: kvnew row j handled by core j % N_CORES
        for j in range(BS):
            if j % N_CORES == c:
                m[j, NT * BS:(NT + 1) * BS] = newcnt[j]
        in_maps.append({
            "k_shard": k_cache[base:base + SLOTS_PER_CORE],
            "v_shard": v_cache[base:base + SLOTS_PER_CORE],
            "k_new16": knew16,
            "v_new16": vnew16,
            "qT": qT,
            "mask": m.astype(ml_dtypes.bfloat16),
            "ident": np.eye(128, dtype=np.float32),
        })
    return in_maps


_PROGS = {}
last_results = None  # BassKernelResults of the most recent kernel() call


def kernel(q, k, v, k_cache, v_cache, slot_mapping, page_indices):
    global last_results
    from concourse.bass_utils import run_bass_kernel_spmd

    in_maps = shard_inputs(q, k, v, k_cache, v_cache, slot_mapping,
                           page_indices)
    if "prog" not in _PROGS:
        try:
            _PROGS["prog"] = build_program(mask_4d=True)
        except Exception:
            _PROGS["prog"] = build_program(mask_4d=False)
    res = run_bass_kernel_spmd(_PROGS["prog"], in_maps,
                               core_ids=list(range(N_CORES)))
    last_results = res

    acc = np.zeros((64, H, 129), dtype=np.float64)
    for c in range(N_CORES):
        acc += res.results[c]["num"].astype(np.float64).reshape(64, H, 129)
    num = acc[:, :, 0:128]                      # [(b,g), h, d]
    den = acc[:, :, 128]                        # [(b,g), h]
    o = num / den[:, :, None]
    o = o.reshape(BS, GROUP, NUM_KV_HEADS, HEAD_DIM)   # [b, g, h, d]
    out = o.transpose(0, 2, 1, 3).reshape(BS, NUM_HEADS * HEAD_DIM)
    return np.ascontiguousarray(out.astype(np.float32))
